# revision 28
# baseline (speedup 1.0000x reference)
"""ChebNet (K=5, 2 conv layers + mean-pool + linear head) on 8 Trainium2
NeuronCores.

Strategy (node sharding, bf16 SpMV path, split-AllGather pipelining):
  - Nodes are split 8 ways; core m owns rows [m*6250, (m+1)*6250).
  - The Chebyshev recurrence runs in Tx-space per core:
        Tx_{k+1} = c1 * gsum_k + c2 * Tx_k - Tx_{k-1}
    where gsum_k[r] = sum_{e: row_e=r} u_k[col_e],  u_k = dis * Tx_k. The
    per-edge Laplacian weight -2*dis_row*dis_col/lam_row factors into the
    per-node scalars c1 = -4*dis^2/lam (row side) and dis (gather side), so
    no per-edge multiply is needed.
  - The u_k table lives in HBM as bf16 rows (256B) in a PERMUTED layout:
    block A = all cores' local rows [0, 3125), block B = local rows
    [3125, 6250). Each round does TWO AllGathers (one per block), so the
    block-B collective of round r overlaps with the block-A gathers +
    segment-sum matmuls of round r+1.
  - The SpMV gather u_k[col] uses gpsimd.dma_gather (256B bf16 rows); the
    segment-sum uses one-hot S-matrix matmuls on TensorE (bf16): edges are
    pre-grouped (host side) into 128-edge chunks, each chunk confined to a
    64-row window and to one source block. Pass A accumulates block-A
    contributions into SBUF as gsA = c1*psum (- Tx_{k-1}); pass B finishes
    Tx_{k+1} = c1*psum_B + gsA.
  - out = sum_k Tx_k @ W[k] accumulates via TensorE with PE-transposed Tx
    tiles (bf16); pooling via one-hot (1/count) matmul + AllReduce.
"""
import sys

sys.path.insert(0, "/opt/trn_rl_repo")

import numpy as np
import ml_dtypes

BF16 = ml_dtypes.bfloat16

# ---------------- problem constants (hardcoded per contract) ----------------
N = 50000
E = 800000
F = 128          # F_IN == hidden == 128
C = 10
S = 5            # Chebyshev order
B = 8            # graphs
NCORES = 8
RPC = N // NCORES          # 6250 rows per core
NT = (RPC + 127) // 128    # 49 node tiles per core (last has 106 rows)
RPC_PAD = NT * 128         # 6272
WIN = 64                   # segment-sum window (psum partition slice)
NWIN = (RPC + WIN - 1) // WIN  # 98 windows per core
WPT = 128 // WIN           # windows per 128-row node tile
NQ = 4                     # SWDGE queues used for gather descgen
PB = RPC // 2              # 3125 local rows per block
NHALF = N // 2             # 25000 table rows per block
GCH = 61                   # chunks per gather call / S-stream group

_CACHE = {}


# ======================= host-side preprocessing ============================

def _prep(x, edge_index, batch, lambda_max):
    row = np.asarray(edge_index[0], dtype=np.int64)
    col = np.asarray(edge_index[1], dtype=np.int64)
    batch = np.asarray(batch, dtype=np.int64)
    lambda_max = np.asarray(lambda_max, dtype=np.float32)
    x = np.asarray(x, dtype=np.float32)

    deg = np.bincount(row, minlength=N).astype(np.float32)
    dis = np.where(deg > 0, 1.0 / np.sqrt(np.maximum(deg, 1e-12)), 0.0).astype(
        np.float32
    )
    lam = lambda_max[batch]                      # [N]
    c1 = (-4.0 * dis / lam).astype(np.float32)
    c2 = (2.0 * (2.0 / lam - 1.0)).astype(np.float32)
    has_diag = bool(np.any(np.abs(c2) > 1e-30))

    # permuted table position: block A = local rows [0, PB), block B = rest
    nn = np.arange(N)
    m_of = nn // RPC
    r_of = nn % RPC
    pos_all = np.where(
        r_of < PB, m_of * PB + r_of, NHALF + m_of * PB + (r_of - PB)
    ).astype(np.int64)

    u0 = np.zeros((N, F), dtype=BF16)            # gather table for round 1
    u0[pos_all] = (x * dis[:, None]).astype(BF16)

    counts = np.bincount(batch, minlength=B).astype(np.float32)
    invcnt = (1.0 / np.maximum(counts, 1.0)).astype(np.float32)

    # ---- per-core edge lists grouped by (window, source block) ----
    order = np.argsort(row, kind="stable")
    row_s, col_s = row[order], col[order]
    pos_s = pos_all[col_s]
    core_lists = []  # [m][half][w] -> (local_rows, table_positions)
    for m in range(NCORES):
        sel = (row_s >= m * RPC) & (row_s < (m + 1) * RPC)
        er = row_s[sel] - m * RPC
        ep = pos_s[sel]
        w_id = er // WIN
        halves = []
        for half in (0, 1):
            hm = (ep < NHALF) if half == 0 else (ep >= NHALF)
            erh, eph, wh = er[hm], ep[hm], w_id[hm]
            cuts = np.searchsorted(wh, np.arange(1, NWIN))
            halves.append(list(zip(np.split(erh, cuts), np.split(eph, cuts))))
        core_lists.append(halves)

    # chunks-per-window, equal across cores (SPMD: one program for all)
    cpw = np.zeros((2, NWIN), dtype=np.int64)
    for half in (0, 1):
        for w in range(NWIN):
            mx = max(len(core_lists[m][half][w][0]) for m in range(NCORES))
            cpw[half, w] = (mx + 127) // 128
    nch = [int(cpw[0].sum()), int(cpw[1].sum())]

    chunk_win = [
        np.repeat(np.arange(NWIN), cpw[h]).astype(np.int64) for h in (0, 1)
    ]

    per_core = []
    for m in range(NCORES):
        idx_arrs, s_arrs = [], []
        for half in (0, 1):
            tot = nch[half]
            cols_pad = np.zeros(tot * 128, dtype=np.int64)
            s_np = np.zeros((128, tot * WIN), dtype=np.float32)
            c0 = 0
            for w in range(NWIN):
                er_w, ep_w = core_lists[m][half][w]
                nck = int(cpw[half, w])
                k = len(er_w)
                base = c0 * 128
                cols_pad[base : base + k] = ep_w - (NHALF if half else 0)
                cc = c0 + np.arange(k) // 128
                pp = np.arange(k) % 128
                s_np[pp, cc * WIN + (er_w - WIN * w)] = 1.0
                c0 += nck
            nidx = tot * 128
            wrapped = np.zeros((16, tot * 8), dtype=np.int16)
            ii = np.arange(nidx)
            wrapped[ii % 16, ii // 16] = cols_pad.astype(np.int16)
            idx_arrs.append(np.tile(wrapped, (8, 1)))
            s_arrs.append(s_np.astype(BF16))

        base = m * RPC
        pad = RPC_PAD - RPC

        def tile_layout(v):  # [N] -> per-core [128, NT]
            vv = np.concatenate(
                [v[base : base + RPC], np.zeros(pad, np.float32)]
            )
            return np.ascontiguousarray(vv.reshape(NT, 128).T).astype(np.float32)

        x_pad = np.concatenate(
            [x[base : base + RPC], np.zeros((pad, F), np.float32)], axis=0
        )
        # state layout [128, NT*128]: st[p, t*128+f] = x[base+t*128+p, f]
        x_state = np.ascontiguousarray(
            x_pad.reshape(NT, 128, F).transpose(1, 0, 2).reshape(128, NT * F)
        )
        spl = np.zeros((128, NT * B), dtype=np.float32)
        lb = batch[base : base + RPC]
        ii = np.arange(RPC)
        spl[ii % 128, (ii // 128) * B + lb] = invcnt[lb]

        per_core.append(
            dict(
                idx_low=idx_arrs[0],
                idx_high=idx_arrs[1],
                S_low=s_arrs[0],
                S_high=s_arrs[1],
                x_state=x_state.astype(BF16),
                c1=tile_layout(c1),
                c1h=tile_layout((c1 * 0.5).astype(np.float32)),
                c2=tile_layout(c2),
                c2h=tile_layout((c2 * 0.5).astype(np.float32)),
                dis=tile_layout(dis),
                spool=spl.astype(BF16),
            )
        )

    meta = dict(nch=nch, chunk_win=chunk_win, cpw=cpw, has_diag=has_diag)
    return meta, per_core, u0


# ========================= device program ===================================

def _patch_tile_context(tile):
    """walrus in this env rejects multi-sync-wait TPB_CTRL ops; split the
    TileContext tail-drain waits across one drain per async proc lane."""

    class SplitDrainTileContext(tile.TileContext):
        def _drain_and_barrier(self, tick_clock, wait_clock):
            from concourse.vector_clock import ScopedClock, VectorClock

            vec = list(tick_clock.global_clock)
            n_procs = len(vec)
            groups = []
            for i, t in enumerate(vec):
                if t > 0:
                    g = [0] * n_procs
                    g[i] = t
                    groups.append(g)
            if not groups:
                groups.append([0] * n_procs)
            for g in groups[:-1]:
                d = self.nc.sync.drain()
                wait_clock.add_sem_waits(d.ins, ScopedClock({None: VectorClock(g)}))
            d = self.nc.sync.drain()
            wait_clock.add_sem_waits(
                d.ins, ScopedClock({None: VectorClock(groups[-1])})
            )
            self.nc.all_engine_barrier()
            popped = self.nc._tile_sem_poison_stack.pop()
            assert popped is self._sem_poison
            self.nc.clear_and_free_semaphores(list(self.sems.allocated().values()))
            self.nc.all_engine_barrier()

    return SplitDrainTileContext


def _build_program(meta):
    import os

    import concourse.bass as bass
    import concourse.mybir as mybir
    from concourse import bacc, tile
    from concourse.alu_op_type import AluOpType

    dbg_rounds = int(os.environ.get("CHEB_DBG_ROUNDS", "99"))
    dbg_ag = os.environ.get("CHEB_DBG_AG", "1") == "1"
    dbg_no_mm = os.environ.get("CHEB_DBG_NO_MM", "0") == "1"
    dbg_no_gather = os.environ.get("CHEB_DBG_NO_GATHER", "0") == "1"
    dbg_no_contrib = os.environ.get("CHEB_DBG_NO_CONTRIB", "0") == "1"
    rounds_left = [dbg_rounds]

    TC = _patch_tile_context(tile)
    f32 = mybir.dt.float32
    bf16 = mybir.dt.bfloat16
    i16 = mybir.dt.int16
    nch = meta["nch"]
    chunk_win = meta["chunk_win"]
    cpw = meta["cpw"]
    has_diag = meta["has_diag"]
    ts = bass.ts

    nc = bacc.Bacc(num_devices=NCORES, num_swdge_queues=NQ)

    # ---------------- I/O ----------------
    dp = nc.declare_dram_parameter
    u0_full = dp("u0_full", [N, F], bf16, isOutput=False)
    idx_low = dp("idx_low", [128, nch[0] * 8], i16, isOutput=False)
    idx_high = dp("idx_high", [128, nch[1] * 8], i16, isOutput=False)
    S_low = dp("S_low", [128, nch[0] * WIN], bf16, isOutput=False)
    S_high = dp("S_high", [128, nch[1] * WIN], bf16, isOutput=False)
    x_state_in = dp("x_state", [128, NT * F], bf16, isOutput=False)
    c1_in = dp("c1", [128, NT], f32, isOutput=False)
    c1h_in = dp("c1h", [128, NT], f32, isOutput=False)
    c2_in = dp("c2", [128, NT], f32, isOutput=False)
    c2h_in = dp("c2h", [128, NT], f32, isOutput=False)
    dis_in = dp("dis", [128, NT], f32, isOutput=False)
    spool_in = dp("spool", [128, NT * B], bf16, isOutput=False)
    W1_in = dp("W1s", [128, S * F], bf16, isOutput=False)
    W2_in = dp("W2s", [128, S * F], bf16, isOutput=False)
    b1_in = dp("b1v", [128, 1], f32, isOutput=False)
    b2_in = dp("b2v", [128, 1], f32, isOutput=False)
    wlin_in = dp("wlin", [128, C], f32, isOutput=False)
    blin_in = dp("blin", [C, 1], f32, isOutput=False)
    ident_in = dp("ident", [128, 128], f32, isOutput=False)
    identb_in = dp("identb", [128, 128], bf16, isOutput=False)
    logits_out = dp("logits", [B, C], f32, isOutput=True)

    ag_bufs = [
        nc.dram_tensor(f"ag{i}", [N, F], bf16, addr_space="Shared")
        for i in range(3)
    ]
    u_shardA = [nc.dram_tensor(f"ushardA{i}", [PB, F], bf16) for i in range(2)]
    u_shardB = [nc.dram_tensor(f"ushardB{i}", [PB, F], bf16) for i in range(2)]
    pooled_bounce = nc.dram_tensor("pooled_bounce", [B, F], f32)
    pooled_shared = nc.dram_tensor("pooled_shared", [B, F], f32, addr_space="Shared")

    rg = [list(range(NCORES))]

    with TC(nc, num_cores=NCORES) as tc:
        with (
            tc.tile_pool(name="const", bufs=1) as cpool,
            tc.tile_pool(name="state", bufs=1) as stp,
            tc.tile_pool(name="msg", bufs=3) as msg_pool,
            tc.tile_pool(name="sstr", bufs=3) as s_pool,
            tc.tile_pool(name="small", bufs=3) as small_pool,
            tc.tile_pool(name="gpsA", bufs=2, space="PSUM") as gpsA_pool,
            tc.tile_pool(name="gpsB", bufs=2, space="PSUM") as gpsB_pool,
            tc.tile_pool(name="tps", bufs=2, space="PSUM") as tps_pool,
            tc.tile_pool(name="aps", bufs=2, space="PSUM") as aps_pool,
        ):
            # ---------- constants ----------
            def load_const(shape, dtype, src, name):
                t = cpool.tile(shape, dtype, name=name, tag=name)
                nc.sync.dma_start(out=t[:], in_=src[:])
                return t

            idxl_t = load_const([128, nch[0] * 8], i16, idx_low, "idxl")
            idxh_t = load_const([128, nch[1] * 8], i16, idx_high, "idxh")
            c1_t = load_const([128, NT], f32, c1_in, "c1t")
            c1h_t = load_const([128, NT], f32, c1h_in, "c1ht")
            c2_t = load_const([128, NT], f32, c2_in, "c2t") if has_diag else None
            c2h_t = load_const([128, NT], f32, c2h_in, "c2ht") if has_diag else None
            dis_t = load_const([128, NT], f32, dis_in, "dist")
            spool_t = load_const([128, NT * B], bf16, spool_in, "spoolt")
            W1_t = load_const([128, S * F], bf16, W1_in, "w1t")
            W2_t = load_const([128, S * F], bf16, W2_in, "w2t")
            b1_t = load_const([128, 1], f32, b1_in, "b1t")
            b2_t = load_const([128, 1], f32, b2_in, "b2t")
            wlin_t = load_const([128, C], f32, wlin_in, "wlint")
            blin_t = load_const([C, 1], f32, blin_in, "blint")
            ident_t = load_const([128, 128], f32, ident_in, "identt")
            identb_t = load_const([128, 128], bf16, identb_in, "identbt")

            # ---------- state ----------
            txA = stp.tile([128, NT * F], bf16, name="txA", tag="txA")
            txB = stp.tile([128, NT * F], bf16, name="txB", tag="txB")
            gsA = stp.tile([128, NT * 128], f32, name="gsA", tag="gsA")
            acc_t = stp.tile([128, NT * 128], f32, name="acc", tag="acc")
            h_t = stp.tile([128, NT * 128], bf16, name="hT", tag="hT")
            nc.sync.dma_start(out=txA[:], in_=x_state_in[:])

            def transpose_tile(src_ap, tag="txT"):
                """[128 p, 128 f] SBUF bf16 -> [128 f, 128 p] SBUF via PE."""
                tp = tps_pool.tile([128, 128], bf16, name="tp", tag="tp")
                nc.tensor.transpose(tp[:], src_ap, identb_t[:])
                sb = small_pool.tile([128, 128], bf16, name="tsb", tag=tag)
                nc.vector.tensor_copy(sb[:], tp[:])
                return sb

            class AccGroup:
                """accumulate per-tile [128,128] psum contributions into
                acc_t, 4 node tiles per psum bank."""

                def __init__(self, k, W_t, first):
                    self.k, self.W_t, self.first = k, W_t, first
                    self.aps = None
                    self.g0 = 0

                def add(self, t, rhsT_ap):
                    if self.aps is None or t - self.g0 >= 4:
                        self.flush(t)
                        self.aps = aps_pool.tile(
                            [128, 512], f32, name="aps", tag="aps"
                        )
                        self.g0 = t
                    nc.tensor.matmul(
                        self.aps[:, ts(t - self.g0, 128)],
                        self.W_t[:, ts(self.k, 128)],
                        rhsT_ap,
                        start=True,
                        stop=True,
                    )

                def flush(self, t_next):
                    if self.aps is None:
                        return
                    gn = t_next - self.g0
                    dst = acc_t[:, self.g0 * 128 : (self.g0 + gn) * 128]
                    if self.first:
                        nc.vector.tensor_copy(dst, self.aps[:, : gn * 128])
                    else:
                        nc.vector.tensor_tensor(
                            dst, self.aps[:, : gn * 128], dst, AluOpType.add
                        )
                    self.aps = None

            def contribution_full(k, src_t, W_t, first, transposed_src):
                """acc (+)= (Tx_k @ W[k])^T from a full-width state tile."""
                ag = AccGroup(k, W_t, first)
                for t in range(NT):
                    if transposed_src:
                        rhs = src_t[:, ts(t, 128)]
                    else:
                        rhs = transpose_tile(src_t[:, ts(t, 128)])[:]
                    ag.add(t, rhs)
                ag.flush(NT)

            ag_rot = [0]

            def allgather_half(shard_buf, half, table_buf):
                """AllGather one block: per-core shard block
                -> table rows [half*NHALF + m*PB ...]."""
                nc.gpsimd.collective_compute(
                    "AllGather",
                    mybir.AluOpType.bypass,
                    replica_groups=rg,
                    ins=[shard_buf[:]],
                    outs=[table_buf[half * NHALF : (half + 1) * NHALF, :]],
                )

            def stage_u_rows(tile_i, src_ap, shardA, shardB):
                us = small_pool.tile([128, 128], bf16, name="us", tag="ustage")
                nc.vector.tensor_scalar_mul(
                    us[:], src_ap, dis_t[:, tile_i : tile_i + 1]
                )
                r0 = tile_i * 128
                rn = min(128, RPC - r0)
                if r0 + rn <= PB:
                    nc.sync.dma_start(out=shardA[r0 : r0 + rn, :], in_=us[:rn, :])
                elif r0 >= PB:
                    nc.sync.dma_start(
                        out=shardB[r0 - PB : r0 - PB + rn, :], in_=us[:rn, :]
                    )
                else:
                    k = PB - r0
                    nc.sync.dma_start(out=shardA[r0:PB, :], in_=us[:k, :])
                    nc.sync.dma_start(out=shardB[0 : rn - k, :], in_=us[k:rn, :])

            nidx_regs = {}

            def nidx_reg(v):
                # shared read-only register per distinct count: a fresh
                # to_reg per gather creates a register WAR hazard that
                # serializes consecutive dma_gathers on the Pool engine.
                if v not in nidx_regs:
                    nidx_regs[v] = nc.gpsimd.to_reg(v)
                return nidx_regs[v]

            qrot = [0]

            def gather_class(half, u_src):
                """issue all dma_gathers + S streams for one source block."""
                tiles = []
                tot = nch[half]
                idx_t = (idxl_t, idxh_t)[half]
                S_src = (S_low, S_high)[half]
                src_ap = u_src[:] if half == 0 else u_src[NHALF:, :]
                for g0 in range(0, tot, GCH):
                    gn = min(GCH, tot - g0)
                    mt = msg_pool.tile([128, GCH, F], bf16, name="mt", tag="m")
                    if dbg_no_gather:
                        nc.vector.memset(mt[:, :gn, :], 0.0)
                    else:
                        nc.gpsimd.dma_gather(
                            out_ap=mt[:, :gn, :],
                            in_ap=src_ap,
                            idxs_ap=idx_t[:, g0 * 8 : (g0 + gn) * 8],
                            num_idxs=gn * 128,
                            num_idxs_reg=nidx_reg(gn * 128),
                            elem_size=F,
                            single_packet=False,
                            queue_num=qrot[0],
                        )
                        qrot[0] = (qrot[0] + 1) % NQ
                    st = s_pool.tile([128, GCH * WIN], bf16, name="st", tag="s")
                    nc.sync.dma_start(
                        out=st[:, : gn * WIN],
                        in_=S_src[:, g0 * WIN : (g0 + gn) * WIN],
                    )
                    tiles.append((mt, st))
                return tiles

            def segsum_tile(half, tiles, ptr, tile_i, ps):
                """accumulate this tile's windows for one source class
                into psum tile ps; returns updated chunk pointer."""
                for wj in range(WPT):
                    w = tile_i * WPT + wj
                    if w >= NWIN:
                        nc.vector.memset(ps[wj * WIN : (wj + 1) * WIN, :], 0.0)
                        continue
                    nck = int(cpw[half][w])
                    if nck == 0 or dbg_no_mm:
                        nc.vector.memset(ps[wj * WIN : (wj + 1) * WIN, :], 0.0)
                        ptr += nck
                        continue
                    for i in range(nck):
                        c = ptr + i
                        g, off = divmod(c, GCH)
                        mt, st = tiles[g]
                        nc.tensor.matmul(
                            ps[wj * WIN : (wj + 1) * WIN, :],
                            st[:, off * WIN : (off + 1) * WIN],
                            mt[:, off, :],
                            start=(i == 0),
                            stop=(i == nck - 1),
                            tile_position=(0, wj * WIN),
                        )
                    ptr += nck
                return ptr

            def spmv_round(r, u_src, tx_prev, tx_cur, tx_out, W_t, do_u,
                           table_out, layer=0):
                if rounds_left[0] <= 0:
                    return
                rounds_left[0] -= 1
                do_u = do_u and dbg_ag
                scope = nc.named_scope(f"L{layer}R{r}")
                scope.__enter__()
                c1x_t = c1h_t if r == 1 else c1_t
                c2x_t = (c2h_t if r == 1 else c2_t) if has_diag else None

                # ---- pass A: block-A gathers + segsum -> gsA ----
                tilesA = gather_class(0, u_src)
                ptr = 0
                for tile_i in range(NT):
                    ps = gpsA_pool.tile([128, 128], f32, name="psA", tag="psA")
                    ptr = segsum_tile(0, tilesA, ptr, tile_i, ps)
                    gdst = gsA[:, ts(tile_i, 128)]
                    sc = c1x_t[:, tile_i : tile_i + 1]
                    if r == 1:
                        if has_diag:
                            # gsA = ps*c1h + c2h*Tx0
                            tmp = small_pool.tile(
                                [128, 128], f32, name="tmp", tag="tmp"
                            )
                            nc.vector.tensor_scalar_mul(
                                tmp[:], tx_cur[:, ts(tile_i, 128)],
                                c2x_t[:, tile_i : tile_i + 1],
                            )
                            nc.vector.scalar_tensor_tensor(
                                gdst, ps[:], sc, tmp[:],
                                AluOpType.mult, AluOpType.add,
                            )
                        else:
                            nc.vector.tensor_scalar_mul(gdst, ps[:], sc)
                    else:
                        if has_diag:
                            tmp = small_pool.tile(
                                [128, 128], f32, name="tmp", tag="tmp"
                            )
                            nc.vector.tensor_scalar_mul(
                                tmp[:], tx_cur[:, ts(tile_i, 128)],
                                c2x_t[:, tile_i : tile_i + 1],
                            )
                            nc.vector.tensor_tensor(
                                tmp[:], tmp[:], tx_prev[:, ts(tile_i, 128)],
                                AluOpType.subtract,
                            )
                            nc.vector.scalar_tensor_tensor(
                                gdst, ps[:], sc, tmp[:],
                                AluOpType.mult, AluOpType.add,
                            )
                        else:
                            # gsA = ps*c1 - Tx_{k-1}
                            nc.vector.scalar_tensor_tensor(
                                gdst, ps[:], sc, tx_prev[:, ts(tile_i, 128)],
                                AluOpType.mult, AluOpType.subtract,
                            )

                # ---- pass B: block-B gathers + segsum + recurrence ----
                tilesB = gather_class(1, u_src)
                accg = AccGroup(r, W_t, first=False)
                ptr = 0
                for tile_i in range(NT):
                    ps = gpsB_pool.tile([128, 128], f32, name="psB", tag="psB")
                    ptr = segsum_tile(1, tilesB, ptr, tile_i, ps)
                    dst = tx_out[:, ts(tile_i, 128)]
                    sc = c1x_t[:, tile_i : tile_i + 1]
                    # dst = ps*c1 + gsA   (gsA already holds c1*psA [- Tx_prev])
                    nc.vector.scalar_tensor_tensor(
                        dst, ps[:], sc, gsA[:, ts(tile_i, 128)],
                        AluOpType.mult, AluOpType.add,
                    )
                    if do_u:
                        stage_u_rows(tile_i, dst, u_shardA[r % 2], u_shardB[r % 2])
                        if tile_i == PB // 128:  # local block A fully staged
                            allgather_half(u_shardA[r % 2], 0, table_out)
                    if not dbg_no_contrib:
                        accg.add(tile_i, transpose_tile(dst)[:])
                accg.flush(NT)
                if do_u:
                    allgather_half(u_shardB[r % 2], 1, table_out)
                scope.__exit__(None, None, None)

            def next_table():
                buf = ag_bufs[ag_rot[0]]
                ag_rot[0] = (ag_rot[0] + 1) % 3
                return buf

            def run_layer(first_src_t, W_t, b_t, u_first, layer=0):
                if first_src_t is None:
                    contribution_full(0, txA, W_t, first=True, transposed_src=False)
                else:
                    contribution_full(
                        0, first_src_t, W_t, first=True, transposed_src=True
                    )
                u_cur = u_first
                for r in range(1, S):
                    do_u = r < S - 1
                    table_out = next_table() if do_u else None
                    spmv_round(
                        r, u_cur,
                        tx_prev=(txA, txB)[(r - 2) % 2] if r >= 2 else None,
                        tx_cur=(txA, txB)[(r - 1) % 2],
                        tx_out=(txA, txB)[r % 2],
                        W_t=W_t,
                        do_u=do_u,
                        table_out=table_out,
                        layer=layer,
                    )
                    if do_u and dbg_ag:
                        u_cur = table_out
                nc.scalar.activation(
                    h_t[:], acc_t[:], mybir.ActivationFunctionType.Relu,
                    bias=b_t[:],
                )

            # ======================= layer 1 =======================
            run_layer(None, W1_t, b1_t, u0_full, layer=1)

            # h -> state layout + u'0 table
            table_h = next_table()
            for t in range(NT):
                hS = transpose_tile(h_t[:, ts(t, 128)], tag="hS")
                nc.vector.tensor_copy(txA[:, ts(t, 128)], hS[:])
                stage_u_rows(t, hS[:], u_shardA[0], u_shardB[0])
                if t == PB // 128 and dbg_ag:
                    allgather_half(u_shardA[0], 0, table_h)
            if dbg_ag:
                allgather_half(u_shardB[0], 1, table_h)
                u_first2 = table_h
            else:
                u_first2 = u0_full

            # ======================= layer 2 =======================
            run_layer(h_t, W2_t, b2_t, u_first2, layer=2)

            # ================== pooling + linear head ==================
            pool_ps = gpsB_pool.tile([B, 128], f32, name="poolps", tag="psB")
            for t in range(NT):
                h2S = transpose_tile(h_t[:, ts(t, 128)], tag="hS")
                nc.tensor.matmul(
                    pool_ps[:],
                    spool_t[:, t * B : (t + 1) * B],
                    h2S[:],
                    start=(t == 0),
                    stop=(t == NT - 1),
                )
            pool_sb = small_pool.tile([B, 128], f32, name="poolsb", tag="poolsb")
            nc.vector.tensor_copy(pool_sb[:], pool_ps[:])
            nc.sync.dma_start(out=pooled_bounce[:], in_=pool_sb[:])
            if dbg_ag:
                nc.gpsimd.collective_compute(
                    "AllReduce", mybir.AluOpType.add, replica_groups=rg,
                    ins=[pooled_bounce[:]], outs=[pooled_shared[:]],
                )
            else:
                nc.sync.dma_start(out=pooled_shared[:], in_=pooled_bounce[:])
            pooled_full = small_pool.tile([B, 128], f32, name="poolfull", tag="poolsb")
            nc.sync.dma_start(out=pooled_full[:], in_=pooled_shared[:])
            ptp = tps_pool.tile([128, B], f32, name="ptp", tag="tp")
            nc.tensor.transpose(ptp[:], pooled_full[:], ident_t[:B, :B])
            pooledT = small_pool.tile([128, B], f32, name="pooledT", tag="poolsb")
            nc.vector.tensor_copy(pooledT[:], ptp[:])
            log_ps = tps_pool.tile([C, B], f32, name="logps", tag="tp")
            nc.tensor.matmul(log_ps[:], wlin_t[:], pooledT[:], start=True, stop=True)
            log_sb = small_pool.tile([C, B], f32, name="logsb", tag="poolsb")
            nc.vector.tensor_scalar_add(log_sb[:], log_ps[:], blin_t[:])
            nc.sync.dma_start(
                out=logits_out.rearrange("a b -> b a")[:, :], in_=log_sb[:]
            )

    nc.finalize()
    return nc


# ============================ entry point ===================================

def kernel(x, edge_index, batch, lambda_max, W1, b1, W2, b2, Wlin, blin):
    from concourse.bass_utils import run_bass_kernel_spmd

    x = np.asarray(x, np.float32)
    W1 = np.asarray(W1, np.float32)
    b1 = np.asarray(b1, np.float32)
    W2 = np.asarray(W2, np.float32)
    b2 = np.asarray(b2, np.float32)
    Wlin = np.asarray(Wlin, np.float32)
    blin = np.asarray(blin, np.float32)

    meta, per_core, u0 = _prep(x, edge_index, batch, lambda_max)

    import os as _os
    key = (
        _os.environ.get("CHEB_DBG_ROUNDS", "99"),
        _os.environ.get("CHEB_DBG_NO_MM", "0"),
        _os.environ.get("CHEB_DBG_NO_GATHER", "0"),
        _os.environ.get("CHEB_DBG_NO_CONTRIB", "0"),
        _os.environ.get("CHEB_DBG_AG", "1"),
        meta["nch"][0],
        meta["nch"][1],
        meta["has_diag"],
        tuple(int(v) for v in meta["cpw"][0]),
        tuple(int(v) for v in meta["cpw"][1]),
    )
    if key not in _CACHE:
        _CACHE.clear()
        _CACHE[key] = _build_program(meta)
    nc = _CACHE[key]

    W1s = np.ascontiguousarray(W1.transpose(1, 0, 2).reshape(F, S * F)).astype(BF16)
    W2s = np.ascontiguousarray(W2.transpose(1, 0, 2).reshape(F, S * F)).astype(BF16)
    common = dict(
        u0_full=u0,
        W1s=W1s,
        W2s=W2s,
        b1v=np.ascontiguousarray(b1.reshape(F, 1)),
        b2v=np.ascontiguousarray(b2.reshape(F, 1)),
        wlin=np.ascontiguousarray(Wlin),
        blin=np.ascontiguousarray(blin.reshape(C, 1)),
        ident=np.eye(128, dtype=np.float32),
        identb=np.eye(128, dtype=np.float32).astype(BF16),
    )
    in_maps = []
    for m in range(NCORES):
        pc = per_core[m]
        in_maps.append(
            dict(
                common,
                idx_low=pc["idx_low"],
                idx_high=pc["idx_high"],
                S_low=pc["S_low"],
                S_high=pc["S_high"],
                x_state=pc["x_state"],
                c1=pc["c1"],
                c1h=pc["c1h"],
                c2=pc["c2"],
                c2h=pc["c2h"],
                dis=pc["dis"],
                spool=pc["spool"],
            )
        )

    res = run_bass_kernel_spmd(nc, in_maps, list(range(NCORES)))
    kernel._last_results = res
    return np.asarray(res.results[0]["logits"], dtype=np.float32)
